# revision 2
# baseline (speedup 1.0000x reference)
"""Trainium2 Bass kernel for a 2-layer GAT (PyG GATConv-style) on 8 NeuronCores.

Strategy (dst-node sharding, per spec sharding_hint):
  - Nodes are partitioned across 8 cores by dst ownership.  Each core
    replicates the layer-1 projection h1 = x @ W1_ext for ALL nodes into a DRAM
    gather table T1 (fp16 rows [h(256)|a_s(4)|a_d(4)|pad] = 384 elems = 768B).
  - Edges (with self-loops) are sorted by dst and grouped into 128-node dst
    windows; per window they are split into two segments by src row so gather
    indices fit dma_gather's int16 (segment B gathers from a rebased table AP).
  - Per 128-edge tile: dma_gather rows by src; one-hot(dstoff) built on DVE via
    is_equal against iota with broadcast APs (pad edges carry dstoff=-1 and thus
    contribute nothing); a_d[dst] expanded per edge via one-hot matmuls;
    ex = exp(leakyrelu(a_s+a_d) - C) emitted pre-replicated by ACT so the
    V = h*ex multiply is a plain 2-operand fp16 DVE op; softmax max-subtraction
    is dropped (mathematically identical, fp16-validated, C keeps exp in range)
    and normalization happens once per window after fp32 PSUM accumulation.
  - Layer 1 runs "flipped" (PSUM [feat, node]) so its ELU'd output appears as
    [256,128] SBUF tiles == the lhsT layout needed to compute layer-2 table
    rows h2 = elu(out1) @ W2_ext per window on the spot.
  - One AllGather ships the per-core layer-2 table to all cores; layer 2
    repeats the edge pipeline in normal orientation and writes [node,64] fp32.
"""

import sys

sys.path.insert(0, "/opt/trn_rl_repo")

import numpy as np

import concourse.bacc as bacc
import concourse.bass as bass
import concourse.mybir as mybir
import concourse.tile as tile
from concourse import bass_utils

F16 = mybir.dt.float16
F32 = mybir.dt.float32
I16 = mybir.dt.int16
OP = mybir.AluOpType
ACT = mybir.ActivationFunctionType

NEG_SLOPE = 0.2
C_SHIFT = 2.0  # global softmax shift: exp(e - C) — cancels in the ratio


def _midb(ap2d, T):
    """[128, X] AP -> [128, T, X] with a broadcast middle dim (free step 0)."""
    aps = [list(d) for d in ap2d.ap]
    return bass.AP(ap2d.tensor, ap2d.offset, [aps[0], [0, T]] + aps[1:])


class Cfg:
    def __init__(self, N, E, NC):
        self.N, self.E, self.NC = N, E, NC
        self.IN, self.HID, self.H, self.OUT = 128, 64, 4, 64
        assert N % NC == 0
        self.LOCAL_N = N // NC
        self.NWIN = -(-self.LOCAL_N // 128)
        self.LOCAL_PAD = self.NWIN * 128
        self.NT1 = -(-N // 128)
        self.NPAD1 = self.NT1 * 128
        self.ROW1 = 384
        self.ROW2 = 128
        self.NROW2 = NC * self.LOCAL_PAD
        self.AR = max(1, min(NC - 1, 32767 // max(self.LOCAL_N, self.LOCAL_PAD)))
        self.ABASE1 = self.AR * self.LOCAL_N
        self.ABASE2 = self.AR * self.LOCAL_PAD
        assert self.ABASE1 <= 32767 and self.ABASE2 <= 32767
        assert self.NPAD1 - self.ABASE1 <= 32767
        assert self.NROW2 - self.ABASE2 <= 32767


class Schedule:
    """Shared (max-over-cores) tile counts + per-core edge orderings."""

    def __init__(self, cfg: Cfg, edge_index: np.ndarray):
        c = cfg
        loop = np.arange(c.N, dtype=np.int32)
        src = np.concatenate([edge_index[0].astype(np.int32), loop])
        dst = np.concatenate([edge_index[1].astype(np.int32), loop])

        per_core = []
        for k in range(c.NC):
            m = (dst // c.LOCAL_N) == k
            s, dl = src[m], dst[m] - k * c.LOCAL_N
            order = np.argsort(dl, kind="stable")
            s, dl = s[order], dl[order]
            w = dl >> 7
            isA = s < c.ABASE1
            wins = []
            for wi in range(c.NWIN):
                mw = w == wi
                mA, mB = mw & isA, mw & ~isA
                wins.append((s[mA], dl[mA] & 127, s[mB], dl[mB] & 127))
            per_core.append(wins)

        self.TA = [max(-(-len(per_core[k][wi][0]) // 128) for k in range(c.NC))
                   for wi in range(c.NWIN)]
        self.TB = [max(-(-len(per_core[k][wi][2]) // 128) for k in range(c.NC))
                   for wi in range(c.NWIN)]
        self.ntiles = sum(self.TA) + sum(self.TB)
        assert max(ta + tb for ta, tb in zip(self.TA, self.TB)) * 128 <= 8192

        self.src1, self.src2, self.dof = [], [], []
        for k in range(c.NC):
            s1l, s2l, dol = [], [], []
            for wi in range(c.NWIN):
                sA, dA, sB, dB = per_core[k][wi]
                for (s_, d_, T, a) in ((sA, dA, self.TA[wi], True),
                                       (sB, dB, self.TB[wi], False)):
                    n = T * 128
                    pad = n - len(s_)
                    padsrc = 0 if a else c.ABASE1
                    s1 = np.concatenate([s_, np.full(pad, padsrc, np.int32)])
                    g2 = (s1 // c.LOCAL_N) * c.LOCAL_PAD + s1 % c.LOCAL_N
                    s1l.append(s1)
                    s2l.append(g2)
                    dol.append(np.concatenate([d_, np.full(pad, -1, np.int32)]))
            self.src1.append(np.concatenate(s1l))
            self.src2.append(np.concatenate(s2l))
            self.dof.append(np.concatenate(dol))


def _wrap_idx(vals: np.ndarray) -> np.ndarray:
    """[n] -> [128, n/16] int16 in dma_gather layout (idx i at partition i%16,
    col i//16; the 16-row block replicated to fill 128 partitions)."""
    n = len(vals)
    a = vals.astype(np.int64).reshape(n // 16, 16).T.astype(np.int16)
    return np.tile(a, (8, 1))


def build_core_inputs(cfg: Cfg, sched: Schedule, inputs: dict) -> list:
    c = cfg
    W1 = inputs["W1"].astype(np.float32)
    as1 = inputs["att_src1"].astype(np.float32)
    ad1 = inputs["att_dst1"].astype(np.float32)
    W2 = inputs["W2"].astype(np.float32)
    as2 = inputs["att_src2"].astype(np.float32)
    ad2 = inputs["att_dst2"].astype(np.float32)

    W1h = W1.reshape(c.IN, c.H, c.HID)
    A_s1 = np.einsum("khc,hc->kh", W1h, as1)
    A_d1 = np.einsum("khc,hc->kh", W1h, ad1)
    w1ext = np.concatenate([W1, A_s1, A_d1], 1).astype(np.float16)  # [128,264]

    A_s2 = (W2 * as2[0][None, :]).sum(1, keepdims=True)
    A_d2 = (W2 * ad2[0][None, :]).sum(1, keepdims=True)
    w2ext = np.concatenate([W2, A_s2, A_d2], 1).astype(np.float16)  # [256, 66]
    w2p = np.concatenate([w2ext[:128], w2ext[128:]], 1)             # [128,132]

    xT = np.zeros((c.IN, c.NPAD1), np.float16)
    xT[:, : c.N] = inputs["x"].astype(np.float16).T

    b1cols = inputs["b1"].astype(np.float32).reshape(2, 128).T.copy()
    b2row = inputs["b2"].astype(np.float32)[None, :]
    iota_rowrep = np.tile(np.arange(128, dtype=np.float16)[None, :], (128, 1))
    b2rep = np.tile(inputs["b2"].astype(np.float32)[None, :], (128, 1))
    iota_col = np.arange(128, dtype=np.float32)[:, None]
    # sel[q, fb*128+f] = 1 iff q == 2*fb + f//64 (PE-side row replication)
    sel = np.zeros((4, 256), np.float32)
    for fb in range(2):
        for f in range(128):
            sel[2 * fb + f // 64, fb * 128 + f] = 1

    shared = dict(w1ext=w1ext, w2p=w2p, xt=xT, b1cols=b1cols,
                  iota_rowrep=iota_rowrep, b2rep=b2rep, iota_col=iota_col,
                  sel=sel)

    maps = []
    nt = sched.ntiles
    for k in range(c.NC):
        s1, s2, dof = sched.src1[k], sched.src2[k], sched.dof[k]
        i1 = np.where(s1 < c.ABASE1, s1, s1 - c.ABASE1)
        i2 = np.where(s2 < c.ABASE2, s2, s2 - c.ABASE2)
        idx1 = np.zeros((128, nt * 8), np.int16)
        idx2 = np.zeros((128, nt * 8), np.int16)
        t0 = 0
        for wi in range(c.NWIN):
            for T in (sched.TA[wi], sched.TB[wi]):
                if T == 0:
                    continue
                sl = slice(t0 * 128, (t0 + T) * 128)
                idx1[:, t0 * 8:(t0 + T) * 8] = _wrap_idx(i1[sl])
                idx2[:, t0 * 8:(t0 + T) * 8] = _wrap_idx(i2[sl])
                t0 += T
        assert t0 == nt

        dofc = dof.reshape(nt, 128).T.astype(np.float16).copy()
        dofr = np.zeros((max(c.NWIN, 1), 8192), np.float16)
        t0 = 0
        for wi in range(c.NWIN):
            n = (sched.TA[wi] + sched.TB[wi]) * 128
            dofr[wi, :n] = dof[t0 * 128:t0 * 128 + n]
            t0 += n // 128

        xown = np.zeros((c.IN, c.LOCAL_PAD), np.float16)
        own = inputs["x"][k * c.LOCAL_N:(k + 1) * c.LOCAL_N]
        xown[:, : c.LOCAL_N] = own.astype(np.float16).T

        m = dict(shared)
        m.update(idx1=idx1, idx2=idx2, dofc=dofc, dofr=dofr, xown=xown)
        maps.append(m)
    return maps


def build_program(nc: bass.Bass, cfg: Cfg, sched: Schedule):
    c = cfg
    nt = sched.ntiles
    NW = c.NWIN

    ap = {}
    for name, shape, dt in [
        ("xt", [c.IN, c.NPAD1], F16), ("xown", [c.IN, c.LOCAL_PAD], F16),
        ("w1ext", [128, 264], F16), ("w2p", [128, 132], F16),
        ("b1cols", [128, 2], F32), ("b2rep", [128, 64], F32),
        ("iota_rowrep", [128, 128], F16), ("iota_col", [128, 1], F32),
        ("sel", [4, 256], F32),
        ("idx1", [128, nt * 8], I16), ("idx2", [128, nt * 8], I16),
        ("dofc", [128, nt], F16), ("dofr", [max(NW, 1), 8192], F16),
    ]:
        ap[name] = nc.dram_tensor(name, shape, dt, kind="ExternalInput").ap()
    ap_out = nc.dram_tensor("out2", [c.LOCAL_PAD, 64], F32,
                            kind="ExternalOutput").ap()

    with tile.TileContext(nc, num_cores=c.NC) as tc:
        _emit(tc, c, sched, ap, ap_out)
    return nc


def _emit(tc, c: Cfg, sched: Schedule, ap, ap_out):
    nc = tc.nc
    nt = sched.ntiles
    NW = c.NWIN

    def t0w(w):
        return sum(sched.TA[:w]) + sum(sched.TB[:w])

    with (
        tc.tile_pool(name="dram", bufs=1, space="DRAM") as dram,
        tc.tile_pool(name="const", bufs=1) as const,
    ):
        T1 = dram.tile([c.NPAD1, c.ROW1], F16)
        T2own = dram.tile([c.LOCAL_PAD, c.ROW2], F16)
        T2full = dram.tile([c.NROW2, c.ROW2], F16)

        def load_const(name, shape, dt):
            t = const.tile(shape, dt, tag=name)
            nc.sync.dma_start(out=t[:], in_=ap[name])
            return t

        w1e = load_const("w1ext", [128, 264], F16)
        w2p = load_const("w2p", [128, 132], F16)
        b1c = load_const("b1cols", [128, 2], F32)
        b2rep = load_const("b2rep", [128, 64], F32)
        irep = load_const("iota_rowrep", [128, 128], F16)
        icol = load_const("iota_col", [128, 1], F32)
        sel = load_const("sel", [4, 256], F32)
        idx1 = load_const("idx1", [128, nt * 8], I16)
        idx2 = load_const("idx2", [128, nt * 8], I16)
        dofc = load_const("dofc", [128, nt], F16)
        adw1 = const.tile([128, 4 * NW], F16)
        adw2 = const.tile([128, NW], F16)
        xown = load_const("xown", [128, c.LOCAL_PAD], F16)
        cshift = const.tile([128, 1], F32)
        nc.vector.memset(cshift[:], -C_SHIFT)
        # one gpsimd register per distinct gather count (register file is small)
        nregs = {}
        for T in sorted(set(sched.TA) | set(sched.TB)):
            if T > 0:
                nregs[T * 128] = nc.gpsimd.to_reg(T * 128)
        tc.strict_bb_all_engine_barrier()

        # ---------------- P1: build T1 ----------------
        G = 3
        with (
            tc.tile_pool(name="p1ps", bufs=2, space="PSUM") as p1ps,
            tc.tile_pool(name="p1sb", bufs=3) as p1sb,
        ):
            for g0 in range(0, c.NT1, G):
                gn = min(G, c.NT1 - g0)
                xchunk = p1sb.tile([128, G * 128], F16, tag="xchunk")
                nc.sync.dma_start(out=xchunk[:, : gn * 128],
                                  in_=ap["xt"][:, g0 * 128:(g0 + gn) * 128])
                ps = p1ps.tile([128, G, 512], F32, tag="p1ps")
                for j in range(gn):
                    nc.tensor.matmul(out=ps[:, j, 0:264],
                                     lhsT=xchunk[:, j * 128:(j + 1) * 128],
                                     rhs=w1e[:], start=True, stop=True)
                stage = p1sb.tile([128, G * c.ROW1], F16, tag="stage")
                nc.gpsimd.memset(
                    stage[:].rearrange("p (g r) -> p g r", g=G)[:, :gn, 264:],
                    0.0)
                src3 = ps[:, :gn, 0:264]
                dst3 = stage[:].rearrange("p (g r) -> p g r", g=G)[:, :gn, :264]
                if (g0 // G) % 2 == 0:
                    nc.vector.tensor_copy(out=dst3, in_=src3)
                else:
                    nc.scalar.copy(out=dst3, in_=src3)
                nc.sync.dma_start(
                    out=T1[g0 * 128:(g0 + gn) * 128, :].rearrange(
                        "(g p) r -> p g r", p=128)[:, :gn, :],
                    in_=stage[:].rearrange("p (g r) -> p g r", g=G)[:, :gn, :])

            # a_d1 for own windows (own range is not 128-grid aligned, so
            # recompute from the per-core xown slice)
            for w in range(NW):
                adps = p1ps.tile([128, 8], F32, tag="adps1")
                nc.tensor.matmul(out=adps[:],
                                 lhsT=xown[:, w * 128:(w + 1) * 128],
                                 rhs=w1e[:, 256:264], start=True, stop=True)
                nc.vector.tensor_copy(out=adw1[:, 4 * w:4 * w + 4],
                                      in_=adps[:, 4:8])

        # ---------------- edge loops ----------------
        with (
            tc.tile_pool(name="g", bufs=2) as gpool,
            tc.tile_pool(name="oh", bufs=2) as ohpool,
            tc.tile_pool(name="ex", bufs=2) as expool,
            tc.tile_pool(name="small", bufs=3) as small,
            tc.tile_pool(name="stg", bufs=3) as stg,
            tc.tile_pool(name="psA", bufs=2, space="PSUM") as psA,
            tc.tile_pool(name="psE", bufs=2, space="PSUM") as psE,
        ):
            def edge_loop(layer):
                t0 = 0
                for w in range(NW):
                    if layer == 1:
                        psN0 = psA.tile([128, 128], F32, tag="psN0")
                        psN1 = psA.tile([128, 128], F32, tag="psN1")
                        psD = psA.tile([4, 128], F32, tag="psD")
                    else:
                        psO = psA.tile([128, 128], F32, tag="psN0",
                                       name="psO")[:, 0:65]
                    Ewin = (sched.TA[w] + sched.TB[w]) * 128
                    dofrep = ohpool.tile([128, Ewin], F16, tag="dofrep")
                    drsrc = bass.AP(ap["dofr"].tensor,
                                    ap["dofr"][w:w + 1, 0:Ewin].offset,
                                    [[0, 128], [1, Ewin]])
                    nc.sync.dma_start(out=dofrep[:], in_=drsrc)
                    mm_i = 0
                    mm_n = (sched.TA[w] + sched.TB[w]) * 128
                    for (T, segA) in ((sched.TA[w], True), (sched.TB[w], False)):
                        if T == 0:
                            continue
                        E = T * 128
                        if layer == 1:
                            tbl = T1[:, :] if segA else T1[c.ABASE1:, :]
                            idx, row = idx1, c.ROW1
                        else:
                            tbl = T2full[:, :] if segA else T2full[c.ABASE2:, :]
                            idx, row = idx2, c.ROW2
                        g = gpool.tile([128, T, row], F16,
                                       tag="gA" if segA else "gB")
                        nc.gpsimd.dma_gather(g[:], tbl,
                                             idx[:, t0 * 8:(t0 + T) * 8],
                                             E, nregs[E], row,
                                             single_packet=False)
                        oh = ohpool.tile([128, T, 128], F16, tag="oh")
                        nc.vector.tensor_tensor(
                            out=oh[:],
                            in0=dofc[:, t0:t0 + T].to_broadcast([128, T, 128]),
                            in1=_midb(irep[:, :], T),
                            op=OP.is_equal)
                        ohT = ohpool.tile([128, T * 128], F16, tag="ohT")
                        e0 = (t0 - t0w(w)) * 128
                        nc.vector.tensor_scalar(
                            out=ohT[:], in0=dofrep[:, e0:e0 + E],
                            scalar1=icol[:, 0:1], scalar2=None,
                            op0=OP.is_equal)
                        hd = 4 if layer == 1 else 1
                        adp = psE.tile([128, T * hd], F32, tag="adp")
                        rhs_ad = adw1[:, 4 * w:4 * w + 4] if layer == 1 \
                            else adw2[:, w:w + 1]
                        for t in range(T):
                            nc.tensor.matmul(
                                out=adp[:, t * hd:(t + 1) * hd],
                                lhsT=ohT[:, t * 128:(t + 1) * 128],
                                rhs=rhs_ad, start=True, stop=True)
                        ea = small.tile([128, T * hd], F32, tag="ea")
                        a_s = g[:, :, 256:260] if layer == 1 \
                            else g[:, :, 64:65]
                        nc.vector.tensor_tensor(
                            out=ea[:].rearrange("p (t h) -> p t h", t=T),
                            in0=a_s,
                            in1=adp[:].rearrange("p (t h) -> p t h", t=T),
                            op=OP.add)
                        pos = small.tile([128, T * hd], F32, tag="pos")
                        nc.vector.tensor_scalar(out=pos[:], in0=ea[:],
                                                scalar1=0.0, scalar2=None,
                                                op0=OP.max)
                        nc.vector.tensor_scalar(out=ea[:], in0=ea[:],
                                                scalar1=0.0,
                                                scalar2=NEG_SLOPE,
                                                op0=OP.min, op1=OP.mult)
                        nc.vector.tensor_tensor(out=ea[:], in0=ea[:],
                                                in1=pos[:], op=OP.add)
                        ex = expool.tile([128, T * hd * 64], F16, tag="ex")
                        nc.scalar.activation(
                            out=ex[:],
                            in_=ea[:].rearrange("p (t h) -> p t h", t=T)
                                .to_broadcast([128, T, hd, 64]),
                            func=ACT.Exp, bias=cshift[:, :])
                        if layer == 2:
                            # compact ex into the (consumed) a_s slot so the
                            # scatter matmul reads [V2 | ex] contiguously
                            nc.scalar.activation(
                                out=g[:, :, 64:65],
                                in_=ea[:].rearrange("p (t h) -> p t h", t=T),
                                func=ACT.Exp, bias=cshift[:, :])
                        nc.vector.tensor_tensor(
                            out=g[:, :, 0:64 * hd], in0=g[:, :, 0:64 * hd],
                            in1=ex[:].rearrange("p (t f) -> p t f", t=T),
                            op=OP.mult)
                        for t in range(T):
                            st, sp = mm_i == 0, mm_i + 128 == mm_n
                            mm_i += 128
                            ohs = oh[:, t, :]
                            if layer == 1:
                                nc.tensor.matmul(out=psN0[:],
                                                 lhsT=g[:, t, 0:128],
                                                 rhs=ohs, start=st, stop=sp)
                                nc.tensor.matmul(out=psN1[:, :],
                                                 lhsT=g[:, t, 128:256],
                                                 rhs=ohs, start=st, stop=sp)
                                exc = ex[:].rearrange(
                                    "p (t h r) -> p t h r", t=T, h=4)[:, t, :, 0]
                                nc.tensor.matmul(out=psD[:, :], lhsT=exc,
                                                 rhs=ohs, start=st, stop=sp)
                            else:
                                nc.tensor.matmul(out=psO[:, 0:65], lhsT=ohs,
                                                 rhs=g[:, t, 0:65],
                                                 start=st, stop=sp)
                        t0 += T
                    # ---------------- window epilogue ----------------
                    if layer == 1:
                        recD = small.tile([4, 128], F32, tag="recD")
                        nc.vector.reciprocal(out=recD[:], in_=psD[:, :])
                        elus = []
                        for fb in range(2):
                            psN = psN0 if fb == 0 else psN1
                            rps = psE.tile([128, 128], F32, tag="adp",
                                           name="rps")
                            nc.tensor.matmul(
                                out=rps[:],
                                lhsT=sel[:, fb * 128:(fb + 1) * 128],
                                rhs=recD[:], start=True, stop=True)
                            rrep = small.tile([128, 128], F32, tag="rrep_sb")
                            nc.vector.tensor_copy(out=rrep[:], in_=rps[:])
                            nrm = small.tile([128, 128], F32, tag="nrm")
                            nc.vector.tensor_tensor(out=nrm[:], in0=psN[:, :],
                                                    in1=rrep[:], op=OP.mult)
                            nc.vector.tensor_scalar(
                                out=nrm[:], in0=nrm[:],
                                scalar1=b1c[:, fb:fb + 1], scalar2=None,
                                op0=OP.add)
                            ex1 = small.tile([128, 128], F32, tag="ex1")
                            nc.scalar.activation(out=ex1[:], in_=nrm[:],
                                                 func=ACT.Exp)
                            nc.scalar.activation(out=ex1[:], in_=ex1[:],
                                                 func=ACT.Relu, scale=-1.0,
                                                 bias=1.0)
                            nc.vector.tensor_scalar(out=nrm[:], in0=nrm[:],
                                                    scalar1=0.0, scalar2=None,
                                                    op0=OP.max)
                            elu = stg.tile([128, 128], F16, tag=f"elu{fb}")
                            nc.vector.tensor_tensor(out=elu[:], in0=nrm[:],
                                                    in1=ex1[:],
                                                    op=OP.subtract)
                            elus.append(elu)
                        if w == NW - 1 and c.LOCAL_N % 128:
                            pad0 = c.LOCAL_N % 128
                            nc.vector.memset(elus[0][:, pad0:], 0.0)
                            nc.vector.memset(elus[1][:, pad0:], 0.0)
                        pst2 = psE.tile([128, 66], F32, tag="adp")
                        nc.tensor.matmul(out=pst2[:], lhsT=elus[0][:],
                                         rhs=w2p[:, 0:66], start=True,
                                         stop=False)
                        nc.tensor.matmul(out=pst2[:], lhsT=elus[1][:],
                                         rhs=w2p[:, 66:132], start=False,
                                         stop=True)
                        t2s = stg.tile([128, 128], F16, tag="t2s")
                        nc.gpsimd.memset(t2s[:, 66:128], 0.0)
                        nc.vector.tensor_copy(out=t2s[:, 0:66], in_=pst2[:])
                        nc.vector.tensor_copy(out=adw2[:, w:w + 1],
                                              in_=pst2[:, 65:66])
                        nc.sync.dma_start(
                            out=T2own[w * 128:(w + 1) * 128, :],
                            in_=t2s[:])
                    else:
                        rec2 = small.tile([128, 1], F32, tag="rec2")
                        nc.vector.reciprocal(out=rec2[:], in_=psO[:, 64:65])
                        o = stg.tile([128, 64], F32, tag="o")
                        nc.vector.tensor_scalar(out=o[:], in0=psO[:, 0:64],
                                                scalar1=rec2[:, :],
                                                scalar2=None, op0=OP.mult)
                        nc.vector.tensor_tensor(out=o[:], in0=o[:],
                                                in1=b2rep[:], op=OP.add)
                        nc.sync.dma_start(
                            out=ap_out[w * 128:(w + 1) * 128, :], in_=o[:])

            import os
            phase = int(os.environ.get("GAT_PHASE", "4"))
            if phase >= 2:
                edge_loop(1)
            if phase >= 3:
                nc.gpsimd.collective_compute(
                    "AllGather", OP.bypass,
                    replica_groups=[list(range(c.NC))],
                    ins=[T2own.opt()], outs=[T2full.opt()])
            if phase >= 4:
                edge_loop(2)
            if phase < 4:
                # touch out2 so the output is defined
                z = stg.tile([128, 64], F32, tag="o")
                nc.vector.memset(z[:], 0.0)
                for w in range(NW):
                    nc.sync.dma_start(out=ap_out[w * 128:(w + 1) * 128, :],
                                      in_=z[:])


def kernel(x, edge_index, W1, att_src1, att_dst1, b1, W2, att_src2, att_dst2,
           b2) -> np.ndarray:
    inputs = dict(x=np.asarray(x), edge_index=np.asarray(edge_index),
                  W1=np.asarray(W1), att_src1=np.asarray(att_src1),
                  att_dst1=np.asarray(att_dst1), b1=np.asarray(b1),
                  W2=np.asarray(W2), att_src2=np.asarray(att_src2),
                  att_dst2=np.asarray(att_dst2), b2=np.asarray(b2))
    cfg = Cfg(N=inputs["x"].shape[0], E=inputs["edge_index"].shape[1], NC=8)
    sched = Schedule(cfg, inputs["edge_index"])
    in_maps = build_core_inputs(cfg, sched, inputs)

    nc = bacc.Bacc("TRN2", target_bir_lowering=False, debug=False,
                   num_devices=cfg.NC)
    build_program(nc, cfg, sched)
    nc.compile()

    import os
    trace = os.environ.get("GAT_TRACE", "0") == "1"
    res = bass_utils.run_bass_kernel_spmd(nc, in_maps,
                                          core_ids=list(range(cfg.NC)),
                                          trace=trace)
    kernel.last_exec_time_ns = res.exec_time_ns
    kernel.last_trace = res.instructions_and_trace
    out = np.concatenate(
        [res.results[k]["out2"][: cfg.LOCAL_N] for k in range(cfg.NC)], 0)
    return out.astype(np.float32)


if __name__ == "__main__":
    from ref_numpy import get_inputs

    inputs = get_inputs()
    out = kernel(**inputs)
    expected = np.load("/tmp/expected_np.npy")
    err = np.abs(out - expected)
    print("abs max err %.3e  rel %.3e" % (err.max(),
                                          err.max() / np.abs(expected).max()))

